# revision 32
# baseline (speedup 1.0000x reference)
"""MinGRU (2-layer) Trainium2 Bass kernel — fp8/bf16 hybrid, chunk-paired.

Problem: B=8, S=4096, D=H=1024.
  layer(inp, W, b): gh = inp @ W.T + b ; gate, hid = split(gh)
    z = sigmoid(gate); a = 1 - z = sigmoid(-gate)
    g = where(hid >= 0, hid + 0.5, sigmoid(hid)) = max(hid + 0.5, sigmoid(hid))
    h_t = a_t * h_{t-1} + z_t * g_t        (h_0 = 0.5)
  out = layer(layer(x, W0, b0), W1, b1)

Sharding: data-parallel over batch, one batch per NeuronCore (8 cores).

Speed levers vs an fp32r baseline (506 us):
  - matmuls in fp8e4 with perf_mode=DoubleRow (2 k-tiles per instruction,
    ~1.5x PE throughput) where the precision budget allows; bf16 (same PE
    speed as fp32r but FWL weight loads + half the SBUF/DMA) elsewhere.
    Gates fp8 in both layers, hidden fp8 in layer 0 only: measured
    end-to-end rel err ~5.7e-3 vs the 2e-2 gate.
  - g = max(hid + bh + 0.5, sigmoid(hid + bh)) exactly (identity with
    relu(x) + min(sigmoid(x), 0.5)), dropping the relu ACT.
  - elementwise chain owned by DVE (g-STT, b=z*g bf16 tensor_tensor at
    2x mode, scan); Scalar does the three sigmoids; GpSimd only issues
    the casting DMAs. Cross-engine hops are software-pipelined over the
    feature-block loop (strict-FIFO queues head-of-line block otherwise).
  - two seq chunks processed per feature block as one FD=1024 tile
    (same per-partition bias across the pair), halving instruction
    counts and amortizing the ~60-170 cycle per-instr engine overheads.
  - rhs / h1 / out packed [128, pair, k, 1024] so each pair moves as one
    large DMA; h1's fp8 copy for the layer-1 gate rhs is made by a
    casting SWDGE DMA on gpsimd (no engine op).
"""
import sys

sys.path.insert(0, "/opt/trn_rl_repo")

import numpy as np
import ml_dtypes
from contextlib import ExitStack

from concourse import bacc, tile, mybir

dt = mybir.dt
Alu = mybir.AluOpType
Act = mybir.ActivationFunctionType
PerfMode = mybir.MatmulPerfMode

B, S, D, H = 8, 4096, 1024, 1024
SC = 512                # matmul N (PSUM bank)
S2 = 2 * SC             # paired free dim for elementwise ops
NP = S // S2            # 4 chunk pairs
NKT = D // 128          # 8 contraction tiles
NFB = H // 128          # 8 feature blocks

# Per-layer dtype config: True = fp8e4 DoubleRow, False = bf16.
GATE_FP8 = (True, True)
HID_FP8 = (True, True)
LIN_ENGINE = "scalar"   # gpsimd cannot access PSUM (BIR verifier)

F8 = ml_dtypes.float8_e4m3
BF16 = ml_dtypes.bfloat16

_cached = {}


def _need_f8_rhs(layer):
    return GATE_FP8[layer] or HID_FP8[layer]


def _need_bf_rhs(layer):
    return not (GATE_FP8[layer] and HID_FP8[layer])


def _build():
    nc = bacc.Bacc("TRN2", target_bir_lowering=False, debug=False, num_devices=8)

    CH2 = NKT * S2  # elements per pair per packed path

    d_xbf = (
        nc.dram_tensor("xbf", [128, NP * CH2], dt.bfloat16, kind="ExternalInput").ap()
        if _need_bf_rhs(0) else None
    )
    d_x8 = (
        nc.dram_tensor("x8", [128, NP * CH2], dt.float8e4, kind="ExternalInput").ap()
        if _need_f8_rhs(0) else None
    )
    d_wg = [
        nc.dram_tensor(
            f"wg{l}", [128, NKT * H], dt.float8e4 if GATE_FP8[l] else dt.bfloat16,
            kind="ExternalInput",
        ).ap()
        for l in range(2)
    ]
    d_wh = [
        nc.dram_tensor(
            f"wh{l}", [128, NKT * H], dt.float8e4 if HID_FP8[l] else dt.bfloat16,
            kind="ExternalInput",
        ).ap()
        for l in range(2)
    ]
    d_bgn = [
        nc.dram_tensor(f"bgn{l}", [128, NFB], dt.float32, kind="ExternalInput").ap()
        for l in range(2)
    ]
    d_bg = [
        nc.dram_tensor(f"bg{l}", [128, NFB], dt.float32, kind="ExternalInput").ap()
        for l in range(2)
    ]
    d_bh = [
        nc.dram_tensor(f"bh{l}", [128, NFB], dt.float32, kind="ExternalInput").ap()
        for l in range(2)
    ]
    d_bh05 = [
        nc.dram_tensor(f"bh05{l}", [128, NFB], dt.float32, kind="ExternalInput").ap()
        for l in range(2)
    ]
    d_out = nc.dram_tensor("out", [128, NP * CH2], dt.bfloat16, kind="ExternalOutput").ap()

    with tile.TileContext(nc) as tc, ExitStack() as ctx:
        cpool = ctx.enter_context(tc.tile_pool(name="const", bufs=1))
        dpool = ctx.enter_context(tc.tile_pool(name="dram", bufs=1, space="DRAM"))
        wpool = ctx.enter_context(tc.tile_pool(name="w", bufs=1))
        rpool = ctx.enter_context(tc.tile_pool(name="rhs", bufs=2))
        tpool = ctx.enter_context(tc.tile_pool(name="tmp", bufs=2))
        hpool = ctx.enter_context(tc.tile_pool(name="h", bufs=2))
        pspool = ctx.enter_context(tc.tile_pool(name="ps", bufs=2, space="PSUM"))

        t_bgn, t_bg, t_bh, t_bh05 = [], [], [], []
        for l in range(2):
            t = cpool.tile([128, NFB], dt.float32, name=f"bgn{l}", tag=f"bgn{l}")
            nc.scalar.dma_start(t[:], d_bgn[l])
            t_bgn.append(t)
            t = cpool.tile([128, NFB], dt.float32, name=f"bg{l}", tag=f"bg{l}")
            nc.scalar.dma_start(t[:], d_bg[l])
            t_bg.append(t)
            t = cpool.tile([128, NFB], dt.float32, name=f"bh{l}", tag=f"bh{l}")
            nc.scalar.dma_start(t[:], d_bh[l])
            t_bh.append(t)
            t = cpool.tile([128, NFB], dt.float32, name=f"bh05{l}", tag=f"bh05{l}")
            nc.scalar.dma_start(t[:], d_bh05[l])
            t_bh05.append(t)

        # DRAM scratch for h1 (layer-0 output / layer-1 input)
        h1bf = dpool.tile([128, NP * CH2], dt.bfloat16, name="h1bf") if _need_bf_rhs(1) else None
        h1f8 = dpool.tile([128, NP * CH2], dt.float8e4, name="h1f8") if _need_f8_rhs(1) else None

        # Resident weights, 3D [128, k, e]
        t_wg = [
            wpool.tile([128, NKT, H], dt.float8e4 if GATE_FP8[l] else dt.bfloat16,
                       name=f"wg{l}")
            for l in range(2)
        ]
        t_wh = [
            wpool.tile([128, NKT, H], dt.float8e4 if HID_FP8[l] else dt.bfloat16,
                       name=f"wh{l}")
            for l in range(2)
        ]

        def load_w(t_w, d_w, k):
            nc.sync.dma_start(t_w[:, k, :], d_w[:, H * k : H * (k + 1)])

        # first k-tiles of layer 0 so PE can start early
        load_w(t_wg[0], d_wg[0], 0)
        load_w(t_wg[0], d_wg[0], 1)
        load_w(t_wh[0], d_wh[0], 0)
        load_w(t_wh[0], d_wh[0], 1)

        def mm_path(ps, w_t, rhs_bf, rhs_f8, i, c2, fp8):
            """One 128x512 output block (pair half c2) over the K=1024 contraction."""
            lo, hi = SC * c2, SC * (c2 + 1)
            if fp8:
                for p in range(NKT // 2):
                    nc.tensor.matmul(
                        ps[:, lo:hi],
                        w_t[:, 2 * p : 2 * p + 2, 128 * i : 128 * (i + 1)],
                        rhs_f8[:, 2 * p : 2 * p + 2, lo:hi],
                        start=(p == 0),
                        stop=(p == NKT // 2 - 1),
                        perf_mode=PerfMode.DoubleRow,
                    )
            else:
                for k in range(NKT):
                    nc.tensor.matmul(
                        ps[:, lo:hi],
                        w_t[:, k, 128 * i : 128 * (i + 1)],
                        rhs_bf[:, k, lo:hi],
                        start=(k == 0),
                        stop=(k == NKT - 1),
                    )

        def do_layer(l, src_bf, src_f8, dst_bf, dst_f8, prefetch=None):
            carry_tile = [None]
            for c in range(NP):
                rhs_bf = rhs_f8 = None
                if src_bf is not None:
                    rhs_bf = rpool.tile([128, NKT, S2], dt.bfloat16,
                                        name=f"rbf{l}", tag=f"rbf{l}", bufs=2)
                    nc.sync.dma_start(rhs_bf[:], src_bf[:, CH2 * c : CH2 * (c + 1)])
                if src_f8 is not None:
                    rhs_f8 = rpool.tile([128, NKT, S2], dt.float8e4,
                                        name=f"rf8{l}", tag=f"rf8{l}", bufs=2)
                    nc.sync.dma_start(rhs_f8[:], src_f8[:, CH2 * c : CH2 * (c + 1)])
                if prefetch is not None:
                    prefetch(c)

                hch = hpool.tile([128, NFB, S2], dt.bfloat16,
                                 name=f"hch{l}", tag=f"hch{l}", bufs=2)
                prev = carry_tile[0]
                # Software-pipelined fb loop (strict-FIFO queues would
                # head-of-line block on cross-engine hops otherwise):
                #   stage i:   matmuls(i), a/s1/lin ACTs(i)
                #   stage i-1: g = max(lin,s1) TT on DVE; u = a*g on Pool
                #   stage i-2: (pool settles)
                #   stage i-3: bneg = u - g TT on DVE; scan(i-3) on DVE
                aq, sq, lq, gq, uq = {}, {}, {}, {}, {}
                for i in range(NFB + 3):
                    if i < NFB:
                        ps_g = pspool.tile([128, S2], dt.float32, name="psg", tag="psg", bufs=2)
                        ps_h = pspool.tile([128, S2], dt.float32, name="psh", tag="psh", bufs=2)
                        for c2 in range(2):
                            mm_path(ps_g, t_wg[l], rhs_bf, rhs_f8, i, c2, GATE_FP8[l])
                            mm_path(ps_h, t_wh[l], rhs_bf, rhs_f8, i, c2, HID_FP8[l])

                        # a = sigmoid(-(gate + bg)); bias column pre-negated
                        a = tpool.tile([128, S2], dt.bfloat16, name="a", tag="a", bufs=5)
                        nc.scalar.activation(
                            a[:], ps_g[:], Act.Sigmoid, bias=t_bgn[l][:, i : i + 1],
                            scale=-1.0,
                        )

                        s1 = tpool.tile([128, S2], dt.bfloat16, name="s1", tag="s1", bufs=2)
                        nc.scalar.activation(
                            s1[:], ps_h[:], Act.Sigmoid, bias=t_bh[l][:, i : i + 1]
                        )
                        # lin = hid + bh + 0.5 (psum -> sbuf bf16)
                        lin = tpool.tile([128, S2], dt.bfloat16, name="lin", tag="lin", bufs=2)
                        if LIN_ENGINE == "gpsimd":
                            from concourse.bass import broadcast_tensor_aps
                            in0b, in1b = broadcast_tensor_aps(
                                ps_h[:], t_bh05[l][:, i : i + 1]
                            )
                            nc.gpsimd.tensor_tensor(lin[:], in0b, in1b, op=Alu.add)
                        else:
                            nc.scalar.activation(
                                lin[:], ps_h[:], Act.Identity,
                                bias=t_bh05[l][:, i : i + 1],
                            )
                        aq[i], sq[i], lq[i] = a, s1, lin
                    if 1 <= i <= NFB:
                        j = i - 1
                        # g = max(hid + bh + 0.5, sigmoid(hid + bh))
                        # (bf16 SBUF tensor_tensor -> DVE 2x mode)
                        g = tpool.tile([128, S2], dt.bfloat16, name="g", tag="g", bufs=4)
                        nc.vector.tensor_tensor(g[:], lq[j][:], sq[j][:], op=Alu.max)
                        gq[j] = g
                        # u = a * g on Pool (SBUF-only mult is Pool-legal)
                        u = tpool.tile([128, S2], dt.bfloat16, name="u", tag="u", bufs=3)
                        nc.gpsimd.tensor_tensor(u[:], aq[j][:], g[:], op=Alu.mult)
                        uq[j] = u
                    if i >= 3:
                        j = i - 3
                        # bneg = u - g = (a-1)*g = -z*g
                        bz = tpool.tile([128, S2], dt.bfloat16, name="bz", tag="bz", bufs=2)
                        nc.vector.tensor_tensor(bz[:], uq[j][:], gq[j][:], op=Alu.subtract)
                        # h = a * h_prev - bneg
                        init = 0.5 if c == 0 else prev[:, j, S2 - 1 : S2]
                        nc.vector.tensor_tensor_scan(
                            hch[:, j, :], aq[j][:], bz[:], init,
                            op0=Alu.mult, op1=Alu.subtract,
                        )
                carry_tile[0] = hch
                if dst_bf is not None:
                    nc.scalar.dma_start(dst_bf[:, CH2 * c : CH2 * (c + 1)], hch[:])
                if dst_f8 is not None:
                    # casting DMA (bf16 -> fp8) via software DGE
                    nc.gpsimd.dma_start(dst_f8[:, CH2 * c : CH2 * (c + 1)], hch[:])

        def prefetch_l0(c):
            if c == 0:
                for k in range(2, NKT):
                    load_w(t_wg[0], d_wg[0], k)
                    load_w(t_wh[0], d_wh[0], k)
            else:
                # stagger layer-1 weights over remaining pairs: 5 + 5 + 6 loads
                ks = {1: [0, 1, 2], 2: [3, 4, 5], 3: [6, 7]}[c]
                for k in ks:
                    load_w(t_wg[1], d_wg[1], k)
                    load_w(t_wh[1], d_wh[1], k)
                if c == 3:
                    for k in range(NKT):
                        pass  # all loaded above

        do_layer(
            0, d_xbf, d_x8,
            h1bf[:] if h1bf is not None else None,
            h1f8[:] if h1f8 is not None else None,
            prefetch=prefetch_l0,
        )
        do_layer(
            1,
            h1bf[:] if h1bf is not None else None,
            h1f8[:] if h1f8 is not None else None,
            d_out, None,
        )

    nc.compile()
    return nc


def _pack_rhs(xT, np_dt):
    # xT: (D, S) fp32 -> [128, pair, k, t] packed, dtype np_dt
    a = xT.reshape(NKT, 128, NP, S2).transpose(1, 2, 0, 3)
    return np.ascontiguousarray(a.reshape(128, -1).astype(np_dt))


def _pack_w(w_rows, np_dt):
    # w_rows: (E=1024, D=1024) fp32 (gate or hidden half of W) -> [128, k, e]
    a = w_rows.T.reshape(NKT, 128, H).transpose(1, 0, 2)
    return np.ascontiguousarray(a.reshape(128, -1).astype(np_dt))


def _bias_cols(bvec):
    return np.ascontiguousarray(bvec.reshape(NFB, 128).T.astype(np.float32))


def _prep_shared(W0, b0, W1, b1):
    m = {}
    for l, (W, b) in enumerate(((W0, b0), (W1, b1))):
        m[f"wg{l}"] = _pack_w(W[:H], F8 if GATE_FP8[l] else BF16)
        m[f"wh{l}"] = _pack_w(W[H:], F8 if HID_FP8[l] else BF16)
        m[f"bgn{l}"] = _bias_cols(-b[:H])
        m[f"bg{l}"] = _bias_cols(b[:H])
        m[f"bh{l}"] = _bias_cols(b[H:])
        m[f"bh05{l}"] = _bias_cols(b[H:] + 0.5)
    return m


def kernel(x, W0, b0, W1, b1):
    from concourse.bass_utils import run_bass_kernel_spmd

    if "nc" not in _cached:
        _cached["nc"] = _build()
    nc = _cached["nc"]

    x = np.asarray(x)
    W0, b0, W1, b1 = (np.asarray(t, np.float32) for t in (W0, b0, W1, b1))
    shared = _prep_shared(W0, b0, W1, b1)
    in_maps = []
    for b in range(B):
        m = dict(shared)
        xT = np.ascontiguousarray(x[b].T.astype(np.float32))
        if _need_bf_rhs(0):
            m["xbf"] = _pack_rhs(xT, BF16)
        if _need_f8_rhs(0):
            m["x8"] = _pack_rhs(xT, F8)
        in_maps.append(m)

    res = run_bass_kernel_spmd(nc, in_maps, core_ids=list(range(B)))
    out = np.empty((B, S, H), np.float32)
    for b in range(B):
        o = np.asarray(res.results[b]["out"]).astype(np.float32)
        # [128, pair, i, t] -> (S, H)
        o = o.reshape(128, NP, NFB, S2).transpose(1, 3, 2, 0).reshape(S, H)
        out[b] = o
    return out


# revision 39
# speedup vs baseline: 1.3093x; 1.3093x over previous
"""MinGRU (2-layer) Trainium2 Bass kernel — fp8/bf16 hybrid, chunk-paired.

Problem: B=8, S=4096, D=H=1024.
  layer(inp, W, b): gh = inp @ W.T + b ; gate, hid = split(gh)
    z = sigmoid(gate); a = 1 - z = sigmoid(-gate)
    g = where(hid >= 0, hid + 0.5, sigmoid(hid)) = max(hid + 0.5, sigmoid(hid))
    h_t = a_t * h_{t-1} + z_t * g_t        (h_0 = 0.5)
  out = layer(layer(x, W0, b0), W1, b1)

Sharding: data-parallel over batch, one batch per NeuronCore (8 cores).

Speed levers vs an fp32r baseline (506 us):
  - matmuls in fp8e4 with perf_mode=DoubleRow (2 k-tiles per instruction,
    ~1.5x PE throughput) where the precision budget allows; bf16 (same PE
    speed as fp32r but FWL weight loads + half the SBUF/DMA) elsewhere.
    Gates fp8 in both layers, hidden fp8 in layer 0 only: measured
    end-to-end rel err ~5.7e-3 vs the 2e-2 gate.
  - g = max(hid + bh + 0.5, sigmoid(hid + bh)) exactly (identity with
    relu(x) + min(sigmoid(x), 0.5)), dropping the relu ACT.
  - elementwise chain owned by DVE (g-STT, b=z*g bf16 tensor_tensor at
    2x mode, scan); Scalar does the three sigmoids; GpSimd only issues
    the casting DMAs. Cross-engine hops are software-pipelined over the
    feature-block loop (strict-FIFO queues head-of-line block otherwise).
  - two seq chunks processed per feature block as one FD=1024 tile
    (same per-partition bias across the pair), halving instruction
    counts and amortizing the ~60-170 cycle per-instr engine overheads.
  - rhs / h1 / out packed [128, pair, k, 1024] so each pair moves as one
    large DMA; h1's fp8 copy for the layer-1 gate rhs is made by a
    casting SWDGE DMA on gpsimd (no engine op).
"""
import sys

sys.path.insert(0, "/opt/trn_rl_repo")

import numpy as np
import ml_dtypes
from contextlib import ExitStack

from concourse import bacc, tile, mybir

dt = mybir.dt
Alu = mybir.AluOpType
Act = mybir.ActivationFunctionType
PerfMode = mybir.MatmulPerfMode

B, S, D, H = 8, 4096, 1024, 1024
SC = 512                # matmul N (PSUM bank)
S2 = 2 * SC             # paired free dim for elementwise ops
NP = S // S2            # 4 chunk pairs
NKT = D // 128          # 8 contraction tiles
NFB = H // 128          # 8 feature blocks

# Per-layer dtype config: True = fp8e4 DoubleRow, False = bf16.
GATE_FP8 = (True, True)
HID_FP8 = (True, True)
LIN_ENGINE = "scalar"   # gpsimd cannot access PSUM (BIR verifier)

F8 = ml_dtypes.float8_e4m3
BF16 = ml_dtypes.bfloat16

_cached = {}


def _need_f8_rhs(layer):
    return GATE_FP8[layer] or HID_FP8[layer]


def _need_bf_rhs(layer):
    return not (GATE_FP8[layer] and HID_FP8[layer])


def _build():
    nc = bacc.Bacc("TRN2", target_bir_lowering=False, debug=False, num_devices=8)

    CH2 = NKT * S2  # elements per pair per packed path

    d_xbf = (
        nc.dram_tensor("xbf", [128, NP * CH2], dt.bfloat16, kind="ExternalInput").ap()
        if _need_bf_rhs(0) else None
    )
    d_x8 = (
        nc.dram_tensor("x8", [128, NP * CH2], dt.float8e4, kind="ExternalInput").ap()
        if _need_f8_rhs(0) else None
    )
    d_wg = [
        nc.dram_tensor(
            f"wg{l}", [128, NKT * H], dt.float8e4 if GATE_FP8[l] else dt.bfloat16,
            kind="ExternalInput",
        ).ap()
        for l in range(2)
    ]
    d_wh = [
        nc.dram_tensor(
            f"wh{l}", [128, NKT * H], dt.float8e4 if HID_FP8[l] else dt.bfloat16,
            kind="ExternalInput",
        ).ap()
        for l in range(2)
    ]
    d_bgn = [
        nc.dram_tensor(f"bgn{l}", [128, NFB], dt.float32, kind="ExternalInput").ap()
        for l in range(2)
    ]
    d_bg = [
        nc.dram_tensor(f"bg{l}", [128, NFB], dt.float32, kind="ExternalInput").ap()
        for l in range(2)
    ]
    d_bh = [
        nc.dram_tensor(f"bh{l}", [128, NFB], dt.float32, kind="ExternalInput").ap()
        for l in range(2)
    ]
    d_bh05 = [
        nc.dram_tensor(f"bh05{l}", [128, NFB], dt.float32, kind="ExternalInput").ap()
        for l in range(2)
    ]
    d_out = nc.dram_tensor("out", [128, NP * CH2], dt.bfloat16, kind="ExternalOutput").ap()

    with tile.TileContext(nc) as tc, ExitStack() as ctx:
        cpool = ctx.enter_context(tc.tile_pool(name="const", bufs=1))
        dpool = ctx.enter_context(tc.tile_pool(name="dram", bufs=1, space="DRAM"))
        wpool = ctx.enter_context(tc.tile_pool(name="w", bufs=1))
        rpool = ctx.enter_context(tc.tile_pool(name="rhs", bufs=2))
        tpool = ctx.enter_context(tc.tile_pool(name="tmp", bufs=2))
        hpool = ctx.enter_context(tc.tile_pool(name="h", bufs=2))
        pspool = ctx.enter_context(tc.tile_pool(name="ps", bufs=2, space="PSUM"))

        t_bgn, t_bg, t_bh, t_bh05 = [], [], [], []
        for l in range(2):
            t = cpool.tile([128, NFB], dt.float32, name=f"bgn{l}", tag=f"bgn{l}")
            nc.scalar.dma_start(t[:], d_bgn[l])
            t_bgn.append(t)
            t = cpool.tile([128, NFB], dt.float32, name=f"bg{l}", tag=f"bg{l}")
            nc.scalar.dma_start(t[:], d_bg[l])
            t_bg.append(t)
            t = cpool.tile([128, NFB], dt.float32, name=f"bh{l}", tag=f"bh{l}")
            nc.scalar.dma_start(t[:], d_bh[l])
            t_bh.append(t)
            t = cpool.tile([128, NFB], dt.float32, name=f"bh05{l}", tag=f"bh05{l}")
            nc.scalar.dma_start(t[:], d_bh05[l])
            t_bh05.append(t)

        # DRAM scratch for h1's bf16 path (layer-1 hidden in bf16 only);
        # the fp8 copy stays SBUF-resident (4 pair tiles, cast in place).
        h1bf = dpool.tile([128, NP * CH2], dt.bfloat16, name="h1bf") if _need_bf_rhs(1) else None

        # Resident weights, 3D [128, k, e]
        t_wg = [
            wpool.tile([128, NKT, H], dt.float8e4 if GATE_FP8[l] else dt.bfloat16,
                       name=f"wg{l}")
            for l in range(2)
        ]
        t_wh = [
            wpool.tile([128, NKT, H], dt.float8e4 if HID_FP8[l] else dt.bfloat16,
                       name=f"wh{l}")
            for l in range(2)
        ]

        def load_w(t_w, d_w, k):
            nc.sync.dma_start(t_w[:, k, :], d_w[:, H * k : H * (k + 1)])

        # first k-tiles of layer 0 so PE can start early
        load_w(t_wg[0], d_wg[0], 0)
        load_w(t_wg[0], d_wg[0], 1)
        load_w(t_wh[0], d_wh[0], 0)
        load_w(t_wh[0], d_wh[0], 1)

        def mm_path(ps, w_t, rhs_bf, rhs_f8, i, c2, fp8):
            """One 128x512 output block (pair half c2) over the K=1024 contraction."""
            lo, hi = SC * c2, SC * (c2 + 1)
            if fp8:
                for p in range(NKT // 2):
                    nc.tensor.matmul(
                        ps[:, lo:hi],
                        w_t[:, 2 * p : 2 * p + 2, 128 * i : 128 * (i + 1)],
                        rhs_f8[:, 2 * p : 2 * p + 2, lo:hi],
                        start=(p == 0),
                        stop=(p == NKT // 2 - 1),
                        perf_mode=PerfMode.DoubleRow,
                    )
            else:
                for k in range(NKT):
                    nc.tensor.matmul(
                        ps[:, lo:hi],
                        w_t[:, k, 128 * i : 128 * (i + 1)],
                        rhs_bf[:, k, lo:hi],
                        start=(k == 0),
                        stop=(k == NKT - 1),
                    )

        def do_layer(l, src_bf, src_f8, dst_bf, dst_f8_tiles, src_f8_tiles=None,
                     prefetch=None):
            carry_tile = [None]
            for c in range(NP):
                rhs_bf = rhs_f8 = None
                if src_bf is not None:
                    rhs_bf = rpool.tile([128, NKT, S2], dt.bfloat16,
                                        name=f"rbf{l}", tag=f"rbf{l}", bufs=2)
                    nc.sync.dma_start(rhs_bf[:], src_bf[:, CH2 * c : CH2 * (c + 1)])
                if src_f8_tiles is not None:
                    rhs_f8 = src_f8_tiles[c]
                elif src_f8 is not None:
                    rhs_f8 = rpool.tile([128, NKT, S2], dt.float8e4,
                                        name=f"rf8{l}", tag=f"rf8{l}", bufs=2)
                    nc.sync.dma_start(rhs_f8[:], src_f8[:, CH2 * c : CH2 * (c + 1)])
                if prefetch is not None:
                    prefetch(c)

                hch = hpool.tile([128, NFB, S2], dt.bfloat16,
                                 name=f"hch{l}", tag=f"hch{l}", bufs=2)
                prev = carry_tile[0]
                # Software-pipelined fb loop (strict-FIFO queues would
                # head-of-line block on cross-engine hops otherwise):
                #   stage i:   matmuls(i), a/s1/lin ACTs(i)
                #   stage i-1: g = max(lin,s1) TT; bneg STT; scan  (all DVE)
                aq, sq, lq = {}, {}, {}
                for i in range(NFB + 1):
                    if i < NFB:
                        ps_g = pspool.tile([128, S2], dt.float32, name="psg", tag="psg", bufs=2)
                        ps_h = pspool.tile([128, S2], dt.float32, name="psh", tag="psh", bufs=2)
                        for c2 in range(2):
                            mm_path(ps_g, t_wg[l], rhs_bf, rhs_f8, i, c2, GATE_FP8[l])
                            mm_path(ps_h, t_wh[l], rhs_bf, rhs_f8, i, c2, HID_FP8[l])

                        # a = sigmoid(-(gate + bg)); bias column pre-negated
                        a = tpool.tile([128, S2], dt.bfloat16, name="a", tag="a", bufs=3)
                        nc.scalar.activation(
                            a[:], ps_g[:], Act.Sigmoid, bias=t_bgn[l][:, i : i + 1],
                            scale=-1.0,
                        )

                        s1 = tpool.tile([128, S2], dt.bfloat16, name="s1", tag="s1", bufs=2)
                        nc.scalar.activation(
                            s1[:], ps_h[:], Act.Sigmoid, bias=t_bh[l][:, i : i + 1]
                        )
                        # lin = hid + bh + 0.5 (psum -> sbuf bf16)
                        lin = tpool.tile([128, S2], dt.bfloat16, name="lin", tag="lin", bufs=2)
                        if LIN_ENGINE == "gpsimd":
                            from concourse.bass import broadcast_tensor_aps
                            in0b, in1b = broadcast_tensor_aps(
                                ps_h[:], t_bh05[l][:, i : i + 1]
                            )
                            nc.gpsimd.tensor_tensor(lin[:], in0b, in1b, op=Alu.add)
                        else:
                            nc.scalar.activation(
                                lin[:], ps_h[:], Act.Identity,
                                bias=t_bh05[l][:, i : i + 1],
                            )
                        aq[i], sq[i], lq[i] = a, s1, lin
                    if 1 <= i <= NFB:
                        j = i - 1
                        # g = max(hid + bh + 0.5, sigmoid(hid + bh))
                        # (bf16 SBUF tensor_tensor -> DVE 2x mode)
                        g = tpool.tile([128, S2], dt.bfloat16, name="g", tag="g", bufs=2)
                        nc.vector.tensor_tensor(g[:], lq[j][:], sq[j][:], op=Alu.max)
                        # bneg = (a - 1) * g = -z*g
                        bz = tpool.tile([128, S2], dt.bfloat16, name="bz", tag="bz", bufs=2)
                        nc.vector.scalar_tensor_tensor(
                            bz[:], aq[j][:], 1.0, g[:], op0=Alu.subtract, op1=Alu.mult
                        )
                        # h = a * h_prev - bneg
                        init = 0.5 if c == 0 else prev[:, j, S2 - 1 : S2]
                        nc.vector.tensor_tensor_scan(
                            hch[:, j, :], aq[j][:], bz[:], init,
                            op0=Alu.mult, op1=Alu.subtract,
                        )
                carry_tile[0] = hch
                if dst_bf is not None:
                    nc.scalar.dma_start(dst_bf[:, CH2 * c : CH2 * (c + 1)], hch[:])
                if dst_f8_tiles is not None:
                    # casting DMA (bf16 -> fp8, SBUF -> SBUF) via software DGE
                    t8 = hpool.tile([128, NKT, S2], dt.float8e4,
                                    name="h18", tag="h18", bufs=NP)
                    nc.gpsimd.dma_start(t8[:], hch[:])
                    dst_f8_tiles.append(t8)

        def prefetch_l0(c):
            if c == 0:
                for k in range(2, NKT):
                    load_w(t_wg[0], d_wg[0], k)
                    load_w(t_wh[0], d_wh[0], k)
            else:
                # stagger layer-1 weights over remaining pairs: 5 + 5 + 6 loads
                ks = {1: [0, 1, 2], 2: [3, 4, 5], 3: [6, 7]}[c]
                for k in ks:
                    load_w(t_wg[1], d_wg[1], k)
                    load_w(t_wh[1], d_wh[1], k)
                if c == 3:
                    for k in range(NKT):
                        pass  # all loaded above

        h18_tiles = [] if _need_f8_rhs(1) else None
        do_layer(
            0, d_xbf, d_x8,
            h1bf[:] if h1bf is not None else None,
            h18_tiles,
            prefetch=prefetch_l0,
        )
        do_layer(
            1,
            h1bf[:] if h1bf is not None else None,
            None,
            d_out, None,
            src_f8_tiles=h18_tiles,
        )

    nc.compile()
    return nc


def _pack_rhs(xT, np_dt):
    # xT: (D, S) fp32 -> [128, pair, k, t] packed, dtype np_dt
    a = xT.reshape(NKT, 128, NP, S2).transpose(1, 2, 0, 3)
    return np.ascontiguousarray(a.reshape(128, -1).astype(np_dt))


def _pack_w(w_rows, np_dt):
    # w_rows: (E=1024, D=1024) fp32 (gate or hidden half of W) -> [128, k, e]
    a = w_rows.T.reshape(NKT, 128, H).transpose(1, 0, 2)
    return np.ascontiguousarray(a.reshape(128, -1).astype(np_dt))


def _bias_cols(bvec):
    return np.ascontiguousarray(bvec.reshape(NFB, 128).T.astype(np.float32))


def _prep_shared(W0, b0, W1, b1):
    m = {}
    for l, (W, b) in enumerate(((W0, b0), (W1, b1))):
        m[f"wg{l}"] = _pack_w(W[:H], F8 if GATE_FP8[l] else BF16)
        m[f"wh{l}"] = _pack_w(W[H:], F8 if HID_FP8[l] else BF16)
        m[f"bgn{l}"] = _bias_cols(-b[:H])
        m[f"bg{l}"] = _bias_cols(b[:H])
        m[f"bh{l}"] = _bias_cols(b[H:])
        m[f"bh05{l}"] = _bias_cols(b[H:] + 0.5)
    return m


def kernel(x, W0, b0, W1, b1):
    from concourse.bass_utils import run_bass_kernel_spmd

    if "nc" not in _cached:
        _cached["nc"] = _build()
    nc = _cached["nc"]

    x = np.asarray(x)
    W0, b0, W1, b1 = (np.asarray(t, np.float32) for t in (W0, b0, W1, b1))
    shared = _prep_shared(W0, b0, W1, b1)
    in_maps = []
    for b in range(B):
        m = dict(shared)
        xT = np.ascontiguousarray(x[b].T.astype(np.float32))
        if _need_bf_rhs(0):
            m["xbf"] = _pack_rhs(xT, BF16)
        if _need_f8_rhs(0):
            m["x8"] = _pack_rhs(xT, F8)
        in_maps.append(m)

    res = run_bass_kernel_spmd(nc, in_maps, core_ids=list(range(B)))
    out = np.empty((B, S, H), np.float32)
    for b in range(B):
        o = np.asarray(res.results[b]["out"]).astype(np.float32)
        # [128, pair, i, t] -> (S, H)
        o = o.reshape(128, NP, NFB, S2).transpose(1, 3, 2, 0).reshape(S, H)
        out[b] = o
    return out


# revision 44
# speedup vs baseline: 1.3759x; 1.0509x over previous
"""MinGRU (2-layer) Trainium2 Bass kernel — fp8/bf16 hybrid, chunk-paired.

Problem: B=8, S=4096, D=H=1024.
  layer(inp, W, b): gh = inp @ W.T + b ; gate, hid = split(gh)
    z = sigmoid(gate); a = 1 - z = sigmoid(-gate)
    g = where(hid >= 0, hid + 0.5, sigmoid(hid)) = max(hid + 0.5, sigmoid(hid))
    h_t = a_t * h_{t-1} + z_t * g_t        (h_0 = 0.5)
  out = layer(layer(x, W0, b0), W1, b1)

Sharding: data-parallel over batch, one batch per NeuronCore (8 cores).

Speed levers vs an fp32r baseline (506 us):
  - matmuls in fp8e4 with perf_mode=DoubleRow (2 k-tiles per instruction,
    ~1.5x PE throughput) where the precision budget allows; bf16 (same PE
    speed as fp32r but FWL weight loads + half the SBUF/DMA) elsewhere.
    Gates fp8 in both layers, hidden fp8 in layer 0 only: measured
    end-to-end rel err ~5.7e-3 vs the 2e-2 gate.
  - g = max(hid + bh + 0.5, sigmoid(hid + bh)) exactly (identity with
    relu(x) + min(sigmoid(x), 0.5)), dropping the relu ACT.
  - elementwise chain owned by DVE (g-STT, b=z*g bf16 tensor_tensor at
    2x mode, scan); Scalar does the three sigmoids; GpSimd only issues
    the casting DMAs. Cross-engine hops are software-pipelined over the
    feature-block loop (strict-FIFO queues head-of-line block otherwise).
  - two seq chunks processed per feature block as one FD=1024 tile
    (same per-partition bias across the pair), halving instruction
    counts and amortizing the ~60-170 cycle per-instr engine overheads.
  - rhs / h1 / out packed [128, pair, k, 1024] so each pair moves as one
    large DMA; h1's fp8 copy for the layer-1 gate rhs is made by a
    casting SWDGE DMA on gpsimd (no engine op).
"""
import sys

sys.path.insert(0, "/opt/trn_rl_repo")

import numpy as np
import ml_dtypes
from contextlib import ExitStack

from concourse import bacc, tile, mybir

dt = mybir.dt
Alu = mybir.AluOpType
Act = mybir.ActivationFunctionType
PerfMode = mybir.MatmulPerfMode

B, S, D, H = 8, 4096, 1024, 1024
SC = 512                # matmul N (PSUM bank)
S2 = 2 * SC             # paired free dim for elementwise ops
NP = S // S2            # 4 chunk pairs
NKT = D // 128          # 8 contraction tiles
NFB = H // 128          # 8 feature blocks

# Per-layer dtype config: True = fp8e4 DoubleRow, False = bf16.
GATE_FP8 = (True, True)
HID_FP8 = (True, True)
LIN_ENGINE = "scalar"   # gpsimd cannot access PSUM (BIR verifier)

F8 = ml_dtypes.float8_e4m3
BF16 = ml_dtypes.bfloat16

_cached = {}


def _need_f8_rhs(layer):
    return GATE_FP8[layer] or HID_FP8[layer]


def _need_bf_rhs(layer):
    return not (GATE_FP8[layer] and HID_FP8[layer])


def _build():
    nc = bacc.Bacc("TRN2", target_bir_lowering=False, debug=False, num_devices=8)

    CH2 = NKT * S2  # elements per pair per packed path

    d_xbf = (
        nc.dram_tensor("xbf", [128, NP * CH2], dt.bfloat16, kind="ExternalInput").ap()
        if _need_bf_rhs(0) else None
    )
    d_x8 = (
        nc.dram_tensor("x8", [128, NP * CH2], dt.float8e4, kind="ExternalInput").ap()
        if _need_f8_rhs(0) else None
    )
    d_wg = [
        nc.dram_tensor(
            f"wg{l}", [128, NKT * H], dt.float8e4 if GATE_FP8[l] else dt.bfloat16,
            kind="ExternalInput",
        ).ap()
        for l in range(2)
    ]
    d_wh = [
        nc.dram_tensor(
            f"wh{l}", [128, NKT * H], dt.float8e4 if HID_FP8[l] else dt.bfloat16,
            kind="ExternalInput",
        ).ap()
        for l in range(2)
    ]
    d_bgn = [
        nc.dram_tensor(f"bgn{l}", [128, NFB], dt.float32, kind="ExternalInput").ap()
        for l in range(2)
    ]
    d_bg = [
        nc.dram_tensor(f"bg{l}", [128, NFB], dt.float32, kind="ExternalInput").ap()
        for l in range(2)
    ]
    d_bh = [
        nc.dram_tensor(f"bh{l}", [128, NFB], dt.float32, kind="ExternalInput").ap()
        for l in range(2)
    ]
    d_bh05 = [
        nc.dram_tensor(f"bh05{l}", [128, NFB], dt.float32, kind="ExternalInput").ap()
        for l in range(2)
    ]
    d_out = nc.dram_tensor("out", [128, NP * CH2], dt.bfloat16, kind="ExternalOutput").ap()

    with tile.TileContext(nc) as tc, ExitStack() as ctx:
        cpool = ctx.enter_context(tc.tile_pool(name="const", bufs=1))
        dpool = ctx.enter_context(tc.tile_pool(name="dram", bufs=1, space="DRAM"))
        wpool = ctx.enter_context(tc.tile_pool(name="w", bufs=1))
        rpool = ctx.enter_context(tc.tile_pool(name="rhs", bufs=2))
        tpool = ctx.enter_context(tc.tile_pool(name="tmp", bufs=2))
        hpool = ctx.enter_context(tc.tile_pool(name="h", bufs=2))
        pspool = ctx.enter_context(tc.tile_pool(name="ps", bufs=2, space="PSUM"))

        t_bgn, t_bg, t_bh, t_bh05 = [], [], [], []
        for l in range(2):
            t = cpool.tile([128, NFB], dt.float32, name=f"bgn{l}", tag=f"bgn{l}")
            nc.scalar.dma_start(t[:], d_bgn[l])
            t_bgn.append(t)
            t = cpool.tile([128, NFB], dt.float32, name=f"bg{l}", tag=f"bg{l}")
            nc.scalar.dma_start(t[:], d_bg[l])
            t_bg.append(t)
            t = cpool.tile([128, NFB], dt.float32, name=f"bh{l}", tag=f"bh{l}")
            nc.scalar.dma_start(t[:], d_bh[l])
            t_bh.append(t)
            t = cpool.tile([128, NFB], dt.float32, name=f"bh05{l}", tag=f"bh05{l}")
            nc.scalar.dma_start(t[:], d_bh05[l])
            t_bh05.append(t)

        # DRAM scratch for h1's bf16 path (layer-1 hidden in bf16 only);
        # the fp8 copy stays SBUF-resident (4 pair tiles, cast in place).
        h1bf = dpool.tile([128, NP * CH2], dt.bfloat16, name="h1bf") if _need_bf_rhs(1) else None

        # Resident weights, 3D [128, k, e]
        t_wg = [
            wpool.tile([128, NKT, H], dt.float8e4 if GATE_FP8[l] else dt.bfloat16,
                       name=f"wg{l}")
            for l in range(2)
        ]
        t_wh = [
            wpool.tile([128, NKT, H], dt.float8e4 if HID_FP8[l] else dt.bfloat16,
                       name=f"wh{l}")
            for l in range(2)
        ]

        def load_w(t_w, d_w, k, eng=None):
            (eng or nc.sync).dma_start(t_w[:, k, :], d_w[:, H * k : H * (k + 1)])

        # first k-tiles of layer 0 on the Scalar ring, in parallel with the
        # first rhs chunk on the Sync ring, so PE can start early
        load_w(t_wg[0], d_wg[0], 0, nc.scalar)
        load_w(t_wg[0], d_wg[0], 1, nc.scalar)
        load_w(t_wh[0], d_wh[0], 0, nc.scalar)
        load_w(t_wh[0], d_wh[0], 1, nc.scalar)

        def mm_path(ps, w_t, rhs_bf, rhs_f8, i, c2, fp8):
            """One 128x512 output block (pair half c2) over the K=1024 contraction."""
            lo, hi = SC * c2, SC * (c2 + 1)
            if fp8:
                for p in range(NKT // 2):
                    nc.tensor.matmul(
                        ps[:, lo:hi],
                        w_t[:, 2 * p : 2 * p + 2, 128 * i : 128 * (i + 1)],
                        rhs_f8[:, 2 * p : 2 * p + 2, lo:hi],
                        start=(p == 0),
                        stop=(p == NKT // 2 - 1),
                        perf_mode=PerfMode.DoubleRow,
                    )
            else:
                for k in range(NKT):
                    nc.tensor.matmul(
                        ps[:, lo:hi],
                        w_t[:, k, 128 * i : 128 * (i + 1)],
                        rhs_bf[:, k, lo:hi],
                        start=(k == 0),
                        stop=(k == NKT - 1),
                    )

        def do_layer(l, src_bf, src_f8, dst_bf, dst_f8_tiles, src_f8_tiles=None,
                     prefetch=None):
            carry_tile = [None]
            for c in range(NP):
                rhs_bf = rhs_f8 = None
                if src_bf is not None:
                    rhs_bf = rpool.tile([128, NKT, S2], dt.bfloat16,
                                        name=f"rbf{l}", tag=f"rbf{l}", bufs=2)
                    nc.sync.dma_start(rhs_bf[:], src_bf[:, CH2 * c : CH2 * (c + 1)])
                if src_f8_tiles is not None:
                    rhs_f8 = src_f8_tiles[c]
                elif src_f8 is not None:
                    rhs_f8 = rpool.tile([128, NKT, S2], dt.float8e4,
                                        name=f"rf8{l}", tag=f"rf8{l}", bufs=2)
                    if l == 0 and c == 0:
                        # split the very first rhs read so the first k-pair
                        # lands quickly and the PE starts ~2us in
                        KH = NKT // 2 * S2
                        nc.sync.dma_start(
                            rhs_f8[:, : NKT // 2, :], src_f8[:, :KH]
                        )
                        nc.sync.dma_start(
                            rhs_f8[:, NKT // 2 :, :], src_f8[:, KH : CH2]
                        )
                    else:
                        nc.sync.dma_start(
                            rhs_f8[:], src_f8[:, CH2 * c : CH2 * (c + 1)]
                        )
                if prefetch is not None:
                    prefetch(c)

                hch = hpool.tile([128, NFB, S2], dt.bfloat16,
                                 name=f"hch{l}", tag=f"hch{l}", bufs=2)
                prev = carry_tile[0]
                # Software-pipelined fb loop (strict-FIFO queues would
                # head-of-line block on cross-engine hops otherwise):
                #   stage i:   matmuls(i), a/s1/lin ACTs(i)
                #   stage i-1: g = max(lin,s1) TT; product; scan  (all DVE)
                use_zact = l == 1 and c < 3
                per_fb_out = dst_bf if (l == 1 and dst_bf is not None) else None
                aq, sq, lq, zq = {}, {}, {}, {}
                for i in range(NFB + 1):
                    if i < NFB:
                        ps_g = pspool.tile([128, S2], dt.float32, name="psg", tag="psg", bufs=2)
                        ps_h = pspool.tile([128, S2], dt.float32, name="psh", tag="psh", bufs=2)
                        for c2 in range(2):
                            mm_path(ps_g, t_wg[l], rhs_bf, rhs_f8, i, c2, GATE_FP8[l])
                            mm_path(ps_h, t_wh[l], rhs_bf, rhs_f8, i, c2, HID_FP8[l])

                        # a = sigmoid(-(gate + bg)); bias column pre-negated
                        a = tpool.tile([128, S2], dt.bfloat16, name="a", tag="a", bufs=3)
                        nc.scalar.activation(
                            a[:], ps_g[:], Act.Sigmoid, bias=t_bgn[l][:, i : i + 1],
                            scale=-1.0,
                        )

                        s1 = tpool.tile([128, S2], dt.bfloat16, name="s1", tag="s1", bufs=2)
                        nc.scalar.activation(
                            s1[:], ps_h[:], Act.Sigmoid, bias=t_bh[l][:, i : i + 1]
                        )
                        # lin = hid + bh + 0.5 (psum -> sbuf bf16)
                        lin = tpool.tile([128, S2], dt.bfloat16, name="lin", tag="lin", bufs=2)
                        if LIN_ENGINE == "gpsimd":
                            from concourse.bass import broadcast_tensor_aps
                            in0b, in1b = broadcast_tensor_aps(
                                ps_h[:], t_bh05[l][:, i : i + 1]
                            )
                            nc.gpsimd.tensor_tensor(lin[:], in0b, in1b, op=Alu.add)
                        else:
                            nc.scalar.activation(
                                lin[:], ps_h[:], Act.Identity,
                                bias=t_bh05[l][:, i : i + 1],
                            )
                        # For a subset of tiles, offload the product factor to
                        # Scalar (z-ACT) so DVE does a cheap 2x TT instead of
                        # the 1x STT — balances Scalar (~236us) vs DVE (~282).
                        z = None
                        if use_zact:
                            z = tpool.tile([128, S2], dt.bfloat16, name="zz",
                                           tag="zz", bufs=2)
                            nc.scalar.activation(
                                z[:], ps_g[:], Act.Sigmoid,
                                bias=t_bg[l][:, i : i + 1],
                            )
                        aq[i], sq[i], lq[i], zq[i] = a, s1, lin, z
                    if 1 <= i <= NFB:
                        j = i - 1
                        # g = max(hid + bh + 0.5, sigmoid(hid + bh))
                        # (bf16 SBUF tensor_tensor -> DVE 2x mode)
                        g = tpool.tile([128, S2], dt.bfloat16, name="g", tag="g", bufs=2)
                        nc.vector.tensor_tensor(g[:], lq[j][:], sq[j][:], op=Alu.max)
                        bz = tpool.tile([128, S2], dt.bfloat16, name="bz", tag="bz", bufs=2)
                        if zq[j] is not None:
                            # b = z*g ; h = a*h + b
                            nc.vector.tensor_tensor(bz[:], zq[j][:], g[:], op=Alu.mult)
                            op1 = Alu.add
                        else:
                            # bneg = (a-1)*g = -z*g ; h = a*h - bneg
                            nc.vector.scalar_tensor_tensor(
                                bz[:], aq[j][:], 1.0, g[:],
                                op0=Alu.subtract, op1=Alu.mult,
                            )
                            op1 = Alu.subtract
                        init = 0.5 if c == 0 else prev[:, j, S2 - 1 : S2]
                        nc.vector.tensor_tensor_scan(
                            hch[:, j, :], aq[j][:], bz[:], init,
                            op0=Alu.mult, op1=op1,
                        )
                        if per_fb_out is not None:
                            nc.sync.dma_start(
                                per_fb_out[:, (CH2 * c + S2 * j) : (CH2 * c + S2 * (j + 1))],
                                hch[:, j, :],
                            )
                carry_tile[0] = hch
                if dst_bf is not None and per_fb_out is None:
                    nc.scalar.dma_start(dst_bf[:, CH2 * c : CH2 * (c + 1)], hch[:])
                if dst_f8_tiles is not None:
                    # casting DMA (bf16 -> fp8, SBUF -> SBUF) via software DGE
                    t8 = hpool.tile([128, NKT, S2], dt.float8e4,
                                    name="h18", tag="h18", bufs=NP)
                    nc.gpsimd.dma_start(t8[:], hch[:])
                    dst_f8_tiles.append(t8)

        def prefetch_l0(c):
            if c == 0:
                for k in range(2, NKT):
                    load_w(t_wg[0], d_wg[0], k)
                    load_w(t_wh[0], d_wh[0], k)
            else:
                # stagger layer-1 weights over remaining pairs: 5 + 5 + 6 loads
                ks = {1: [0, 1, 2], 2: [3, 4, 5], 3: [6, 7]}[c]
                for k in ks:
                    load_w(t_wg[1], d_wg[1], k)
                    load_w(t_wh[1], d_wh[1], k)
                if c == 3:
                    for k in range(NKT):
                        pass  # all loaded above

        h18_tiles = [] if _need_f8_rhs(1) else None
        do_layer(
            0, d_xbf, d_x8,
            h1bf[:] if h1bf is not None else None,
            h18_tiles,
            prefetch=prefetch_l0,
        )
        do_layer(
            1,
            h1bf[:] if h1bf is not None else None,
            None,
            d_out, None,
            src_f8_tiles=h18_tiles,
        )

    nc.compile()
    return nc


def _pack_rhs(xT, np_dt):
    # xT: (D, S) fp32 -> [128, pair, k, t] packed, dtype np_dt
    a = xT.reshape(NKT, 128, NP, S2).transpose(1, 2, 0, 3)
    return np.ascontiguousarray(a.reshape(128, -1).astype(np_dt))


def _pack_w(w_rows, np_dt):
    # w_rows: (E=1024, D=1024) fp32 (gate or hidden half of W) -> [128, k, e]
    a = w_rows.T.reshape(NKT, 128, H).transpose(1, 0, 2)
    return np.ascontiguousarray(a.reshape(128, -1).astype(np_dt))


def _bias_cols(bvec):
    return np.ascontiguousarray(bvec.reshape(NFB, 128).T.astype(np.float32))


def _prep_shared(W0, b0, W1, b1):
    m = {}
    for l, (W, b) in enumerate(((W0, b0), (W1, b1))):
        m[f"wg{l}"] = _pack_w(W[:H], F8 if GATE_FP8[l] else BF16)
        m[f"wh{l}"] = _pack_w(W[H:], F8 if HID_FP8[l] else BF16)
        m[f"bgn{l}"] = _bias_cols(-b[:H])
        m[f"bg{l}"] = _bias_cols(b[:H])
        m[f"bh{l}"] = _bias_cols(b[H:])
        m[f"bh05{l}"] = _bias_cols(b[H:] + 0.5)
    return m


def kernel(x, W0, b0, W1, b1):
    from concourse.bass_utils import run_bass_kernel_spmd

    if "nc" not in _cached:
        _cached["nc"] = _build()
    nc = _cached["nc"]

    x = np.asarray(x)
    W0, b0, W1, b1 = (np.asarray(t, np.float32) for t in (W0, b0, W1, b1))
    shared = _prep_shared(W0, b0, W1, b1)
    in_maps = []
    for b in range(B):
        m = dict(shared)
        xT = np.ascontiguousarray(x[b].T.astype(np.float32))
        if _need_bf_rhs(0):
            m["xbf"] = _pack_rhs(xT, BF16)
        if _need_f8_rhs(0):
            m["x8"] = _pack_rhs(xT, F8)
        in_maps.append(m)

    res = run_bass_kernel_spmd(nc, in_maps, core_ids=list(range(B)))
    out = np.empty((B, S, H), np.float32)
    for b in range(B):
        o = np.asarray(res.results[b]["out"]).astype(np.float32)
        # [128, pair, i, t] -> (S, H)
        o = o.reshape(128, NP, NFB, S2).transpose(1, 3, 2, 0).reshape(S, H)
        out[b] = o
    return out
